# revision 26
# baseline (speedup 1.0000x reference)
"""Trainium2 Bass kernel for a pre-norm transformer encoder block (v2).

Problem shapes (hardcoded): x [4, 2048, 768], 12 heads x 64, d_ff 3072.

Sharding: 8 cores, no collectives. Core c handles batch b = c // 2 and the
token half h = c % 2 (1024 "own" tokens). Each core receives the full 2048
tokens of its batch (own half first) so it can compute K/V locally; Q and
everything downstream (proj, MLP, output) run on its 1024 own tokens only.

v2 schedule (vs v1 phase-serial; ~1.5x target):
  - LayerNorm rstd via DVE Newton iteration (no ScalarE sqrt -> no activation
    table switches against exp/gelu; safe because var(x) ~ 1 here).
  - h -> h^T via DMA xbar transpose (SBUF->SBUF), PE transposes removed.
    hT layout [P, tile, CT, 128] so each transposed tile lands contiguous.
  - QKV in 4 token-groups of 512; attention chunk-0 head-pair-0 pass is
    interleaved into groups 1..3 (kt tiles consumed as K/V complete).
  - attention per 512-query chunk: 6 passes (1 head pair each) over 16 kt:
    S pair (PE row groups 0/64), exp [128,1024] on ScalarE, PV accumulate
    with the ones-column trick for softmax sums. PV emission is one kt
    behind exp so the PE never queues behind a pending exp.
    Optional fp8e4 DoubleRow PV (pt/V fp8, exp scaled 1/16 to stay < 240,
    two kt tiles contracted per matmul).
  - pipeline: attn(c0) -> [mlp-head(c0); {attn(c1, pair p); fc1(c0, 4f)}x6;
    gelu(c0); fc2(c0)] -> mlp(c1). ScalarE exp of chunk 1 overlaps chunk 0's
    MLP matmuls.
  - fc1 psum staged to SBUF f16 via tensor_scalar(+b1); ONE batched gelu per
    chunk (2 activation-table switches per chunk total).
"""

import os
import sys
import types

import numpy as np

# This image's antenv lacks ``axon_hooks``, so the boot shim can't register
# the NTFF-profiling hook and trace=True silently degrades. Provide the
# registry module with a lazily-built ctypes hook against libaxon_pjrt.so.
if "antenv.axon_hooks" not in sys.modules:
    _m = types.ModuleType("antenv.axon_hooks")
    _m._hook = None

    def _build_ctypes_hook():
        import contextlib
        import ctypes

        so_path = "/opt/axon/libaxon_pjrt.so"
        if not os.path.exists(so_path):
            return None
        lib = ctypes.CDLL(so_path)
        if not hasattr(lib, "axon_start_nrt_profile"):
            return None
        lib.axon_start_nrt_profile.argtypes = [
            ctypes.POINTER(ctypes.c_int64), ctypes.c_size_t]
        lib.axon_start_nrt_profile.restype = ctypes.c_int64
        lib.axon_stop_nrt_profile.argtypes = [ctypes.c_char_p]
        lib.axon_stop_nrt_profile.restype = ctypes.c_int64

        @contextlib.contextmanager
        def _hook(output_dir, device_ids):
            import jax
            jax.devices()
            if device_ids:
                ids = (ctypes.c_int64 * len(device_ids))(*device_ids)
                rc = lib.axon_start_nrt_profile(ids, len(device_ids))
            else:
                rc = lib.axon_start_nrt_profile(None, 0)
            if rc != 0:
                raise RuntimeError(f"axon_start_nrt_profile rc={rc}")
            try:
                yield
            finally:
                n = lib.axon_stop_nrt_profile(str(output_dir).encode())
                if n < 0:
                    raise RuntimeError(f"axon_stop_nrt_profile rc={n}")
                print(f"profile: {n} file(s) written to {output_dir}")

        return _hook

    def _set(h, _m=_m):
        _m._hook = h

    def _get(_m=_m):
        if _m._hook is None:
            _m._hook = _build_ctypes_hook()
        return _m._hook

    _m.set_axon_ntff_profile_hook = _set
    _m.get_axon_ntff_profile_hook = _get
    sys.modules["antenv.axon_hooks"] = _m

B, N, C = 4, 2048, 768
HEADS, HD = 12, 64
FF = 4 * C
P = 128
NT = N // P            # 16 token tiles (full context)
QT_ = (N // 2) // P    # 8 own token tiles
CT = C // P            # 6 feature tiles
FT = FF // P           # 24 ff tiles
NPAIR = HEADS // 2     # 6 head pairs (= CT: 128 features per pair)
LN_EPS = 1e-5

USE_FP8_PV = os.environ.get("KERNEL_FP8_PV", "1") == "1"
EXP_FP8_BIAS = -2.772588722239781  # -ln(16): keeps exp output <= ~19 << 240

_CACHE = {}
LAST_RESULT = None


def _build(has_bpo, has_bo, fp8_pv):
    import concourse.bass as bass
    import concourse.mybir as mybir
    import concourse.tile as tile
    from concourse import bacc
    from contextlib import ExitStack

    F32 = mybir.dt.float32
    F16 = mybir.dt.float16
    FP8 = mybir.dt.float8e4
    AF = mybir.ActivationFunctionType
    OP = mybir.AluOpType
    DR = mybir.MatmulPerfMode.DoubleRow

    nc = bacc.Bacc(None, target_bir_lowering=False)

    # ---- DRAM tensors ----
    x_in = nc.dram_tensor("x_in", [N, C], F32, kind="ExternalInput")
    wq = nc.dram_tensor("wq", [CT, P, CT, P], F16, kind="ExternalInput")
    wk = nc.dram_tensor("wk", [CT, P, CT, P], F16, kind="ExternalInput")
    wv = nc.dram_tensor("wv", [CT, P, C], F16, kind="ExternalInput")
    wp = nc.dram_tensor("wp", [CT, P, C], F16, kind="ExternalInput")
    w1 = nc.dram_tensor("w1", [FT, P, CT, P], F16, kind="ExternalInput")
    w2 = nc.dram_tensor("w2", [FT, P, C], F16, kind="ExternalInput")
    qb = nc.dram_tensor("qb", [P, CT], F32, kind="ExternalInput")
    b1v = nc.dram_tensor("b1v", [P, FT], F32, kind="ExternalInput")
    bpo = nc.dram_tensor("bpo", [C], F32, kind="ExternalInput")
    bo = nc.dram_tensor("bo", [C], F32, kind="ExternalInput")
    onesc = nc.dram_tensor("onesc", [P, NT * HEADS], F16, kind="ExternalInput")
    y = nc.dram_tensor("y", [N // 2, C], F32, kind="ExternalOutput")

    def bcast_rows(t):
        return bass.AP(tensor=t.tensor, offset=t.offset, ap=[[0, P], list(t.ap[0])])

    with tile.TileContext(nc) as tc, ExitStack() as top:
        consts = top.enter_context(tc.tile_pool(name="consts", bufs=1))
        t_qb = consts.tile([P, CT], F32)
        t_b1 = consts.tile([P, FT], F32)
        t_eps = consts.tile([P, 1], F32)
        nc.vector.memset(t_eps[:], LN_EPS)
        t_eb = consts.tile([P, 1], F32)
        nc.vector.memset(t_eb[:], EXP_FP8_BIAS)
        t_bpo = t_bo = None
        if has_bpo:
            t_bpo = consts.tile([P, C], F32)
        if has_bo:
            t_bo = consts.tile([P, C], F32)

        # ---- persistent SBUF state ----
        s_kqv = ExitStack()   # KT/QT/V: freed after attention c1
        s_hT = ExitStack()    # hT: freed after QKV
        s_big = ExitStack()   # OT/xo/h2T/g: until end
        top.enter_context(s_big)

        pool_kqv = s_kqv.enter_context(tc.tile_pool(name="kqv", bufs=1, side="right"))
        t_KT = pool_kqv.tile([P, NPAIR, N], F16)       # K^T feature-major
        t_QT = pool_kqv.tile([P, NPAIR, N // 2], F16)  # Q^T own tokens
        if fp8_pv:
            # [P, kt-pair, j, head, 68]: DoubleRow lhsT; col 64 = ones
            t_V = pool_kqv.tile([P, NT // 2, 2, HEADS, 68], FP8)
        else:
            t_V = pool_kqv.tile([P, NT, HEADS, HD + 1], F16)

        pool_hT = s_hT.enter_context(tc.tile_pool(name="hT", bufs=1, side="right"))
        t_hT = pool_hT.tile([P, NT, CT, P], F16)

        pool_big = s_big.enter_context(tc.tile_pool(name="big", bufs=1))
        t_OT = pool_big.tile([P, 2, NPAIR, 512], F16)   # O^T per chunk
        t_xo = pool_big.tile([P, 2, 4, C], F32)         # residual accum
        t_h2T = pool_big.tile([P, 2, 4, CT, P], F16)
        t_g = None  # fc1/gelu staging; allocated after hT frees its space

        wpool = top.enter_context(tc.tile_pool(name="wlong", bufs=1))
        t_wp = wpool.tile([P, CT, C], F16)

        # ---- PSUM pools (8 banks total: psA 2 + psS 4 + psO 2) ----
        psA = top.enter_context(tc.tile_pool(name="psA", bufs=2, space="PSUM"))
        psS = top.enter_context(tc.tile_pool(name="psS", bufs=2, space="PSUM"))
        psO = top.enter_context(tc.tile_pool(name="psO", bufs=1, space="PSUM"))

        ptp = top.enter_context(tc.tile_pool(name="pt", bufs=3))
        rbp = top.enter_context(tc.tile_pool(name="rb", bufs=1))

        nc.sync.dma_start(t_qb[:], qb[:])
        nc.sync.dma_start(t_b1[:], b1v[:])
        if has_bpo:
            nc.sync.dma_start(t_bpo[:], bcast_rows(bpo[:]))
        if has_bo:
            nc.sync.dma_start(t_bo[:], bcast_rows(bo[:]))

        def rsqrt_newton(pool, var_ap, out, n, iters):
            # out [P, n] f32 = 1/sqrt(var + eps). Newton from y0 = 1/(var+eps)
            # converges monotonically from below for var+eps > 1/3 (true here:
            # LN inputs have variance ~1).
            v = pool.tile([P, n], F32, tag=f"lnv{n}")
            nc.vector.tensor_scalar(
                out=v[:], in0=var_ap, scalar1=t_eps[:, 0:1], scalar2=None,
                op0=OP.add)
            nc.vector.reciprocal(out=out, in_=v[:])
            t = pool.tile([P, n], F32, tag=f"lnt{n}")
            for _ in range(iters):
                nc.vector.tensor_tensor(out=t[:], in0=out, in1=out, op=OP.mult)
                nc.vector.tensor_tensor(out=t[:], in0=t[:], in1=v[:], op=OP.mult)
                nc.vector.tensor_scalar(
                    out=t[:], in0=t[:], scalar1=-0.5, scalar2=1.5,
                    op0=OP.mult, op1=OP.add)
                nc.vector.tensor_tensor(out=out, in0=out, in1=t[:], op=OP.mult)

        def ln_stats(pool, xt, mvb, i):
            stats = pool.tile([P, 3, nc.vector.BN_STATS_DIM], F32, tag="ln_stats")
            for sg in range(3):
                nc.vector.bn_stats(out=stats[:, sg], in_=xt[:, sg * 256:(sg + 1) * 256])
            nc.vector.bn_aggr(out=mvb[:, i], in_=stats[:])

        def ln_apply(xt, mvb, i, rstdb, ht):
            with nc.allow_low_precision(reason="fp16 for matmul input"):
                nc.vector.tensor_scalar(
                    out=ht[:], in0=xt[:], scalar1=mvb[:, i, 0:1],
                    scalar2=rstdb[:, i:i + 1], op0=OP.subtract, op1=OP.mult)

        # ---------------- attention pass (one head pair) ----------------
        def attn_pass(chunk, pair, kts, start, stop, state):
            qs = slice(chunk * 512, (chunk + 1) * 512)
            if start:
                state["pso"] = [
                    psO.tile([HD + 1, 512], F32, tag=f"o{sub}",
                             name=f"pso{sub}_{chunk}_{pair}")
                    for sub in range(2)
                ]
                state["pend"] = None
            pso = state["pso"]

            def emit_pv_f16(kt, pt):
                for sub in range(2):
                    nc.tensor.matmul(
                        pso[sub][:], t_V[:, kt, 2 * pair + sub, :],
                        pt[:, sub * 512:(sub + 1) * 512],
                        start=(kt == 0), stop=(kt == NT - 1))

            def emit_pv_fp8(kp, pt8):
                for sub in range(2):
                    nc.tensor.matmul(
                        pso[sub][:],
                        t_V[:, kp, :, 2 * pair + sub, 0:HD + 1],
                        pt8[:, :, sub * 512:(sub + 1) * 512],
                        start=(kp == 0), stop=(kp == NT // 2 - 1),
                        perf_mode=DR)

            for kt in kts:
                ps = psS.tile([P, 1024], F32, tag="s")
                for sub in range(2):
                    off = sub * HD
                    nc.tensor.matmul(
                        ps[:, sub * 512:(sub + 1) * 512],
                        t_KT[off:off + HD, pair, kt * P:(kt + 1) * P],
                        t_QT[off:off + HD, pair, qs], start=True, stop=True)
                if fp8_pv:
                    j = kt % 2
                    if j == 0:
                        state["pt8"] = ptp.tile(
                            [P, 2, 1024], FP8, tag="pt8",
                            name=f"pt8_{chunk}_{pair}_{kt}")
                    pt8 = state["pt8"]
                    nc.scalar.activation(
                        out=pt8[:, j], in_=ps[:], func=AF.Exp, scale=0.125,
                        bias=t_eb[:])
                    if j == 1:
                        if state["pend"] is not None:
                            emit_pv_fp8(*state["pend"])
                        state["pend"] = (kt // 2, pt8)
                else:
                    pt = ptp.tile([P, 1024], F16, tag="pt")
                    nc.scalar.activation(
                        out=pt[:], in_=ps[:], func=AF.Exp, scale=0.125)
                    if state["pend"] is not None:
                        emit_pv_f16(*state["pend"])
                    state["pend"] = (kt, pt)

            if stop:
                if state["pend"] is not None:
                    if fp8_pv:
                        emit_pv_fp8(*state["pend"])
                    else:
                        emit_pv_f16(*state["pend"])
                    state["pend"] = None
                for sub in range(2):
                    off = sub * HD
                    sums = rbp.tile([1, 512], F32, tag="sums")
                    nc.vector.tensor_copy(out=sums[:], in_=pso[sub][HD:HD + 1, :])
                    r32 = rbp.tile([1, 512], F32, tag="r32")
                    nc.vector.reciprocal_approx_fast(out=r32[:], in_=sums[:])
                    rb = rbp.tile([HD, 512], F32, tag="rb")
                    nc.gpsimd.partition_broadcast(rb[:], r32[:])
                    with nc.allow_low_precision(reason="fp16 matmul input"):
                        nc.vector.tensor_tensor(
                            out=t_OT[off:off + HD, chunk, pair, :],
                            in0=pso[sub][:HD, :], in1=rb[:], op=OP.mult)

        # -------- QKV over 4 token groups (+ attn c0 pair 0 interleaved) ----
        with ExitStack() as qkv_sec:
            lnx = qkv_sec.enter_context(tc.tile_pool(name="lnx", bufs=6))
            lnp = qkv_sec.enter_context(tc.tile_pool(name="ln1", bufs=2))
            htp = qkv_sec.enter_context(tc.tile_pool(name="htp", bufs=2))
            wst = qkv_sec.enter_context(tc.tile_pool(name="wst", bufs=4))
            wvp = qkv_sec.enter_context(tc.tile_pool(name="wv", bufs=1))

            xts0 = []
            for t in range(4):
                xt = lnx.tile([P, C], F32, tag="xt", name=f"xt0_{t}")
                nc.sync.dma_start(xt[:], x_in[t * P:(t + 1) * P, :])
                xts0.append(xt)
            t_wv = wvp.tile([P, CT, C], F16, tag="wv")
            nc.sync.dma_start(t_wv[:], wv[:].rearrange("c p n -> p c n"))
            if fp8_pv:
                for kp in range(NT // 2):
                    for j in range(2):
                        nc.gpsimd.memset(t_V[:, kp, j, :, HD:HD + 1], 1.0)
            else:
                nc.sync.dma_start(
                    t_V[:, :, :, HD:HD + 1],
                    onesc[:].rearrange("p (t h) -> p t h", t=NT)[:, :, :, None])

            c0p0 = {}  # attention chunk-0 pair-0 state (split across groups)

            def v_tile(t):
                for nc2 in range(2):
                    ps = psA.tile([P, 512], F32, tag="mm")
                    for c in range(CT):
                        nc.tensor.matmul(
                            ps[:, 0:384], t_hT[:, t, c, :],
                            t_wv[:, c, nc2 * 384:(nc2 + 1) * 384],
                            start=(c == 0), stop=(c == CT - 1))
                    with nc.allow_low_precision(reason="fp16/fp8"):
                        if fp8_pv:
                            nc.vector.tensor_copy(
                                out=t_V[:, t // 2, t % 2,
                                        6 * nc2:6 * nc2 + 6, 0:HD],
                                in_=ps[:, 0:384].rearrange(
                                    "p (h d) -> p h d", d=HD))
                        else:
                            nc.vector.tensor_copy(
                                out=t_V[:, t, 6 * nc2:6 * nc2 + 6, :HD],
                                in_=ps[:, 0:384].rearrange(
                                    "p (h d) -> p h d", d=HD))

            for g in range(4):  # token groups of 512
                tiles = list(range(4 * g, 4 * g + 4))
                mvb = lnp.tile([P, 4, nc.vector.BN_AGGR_DIM], F32, tag="mvb")
                rstdb = lnp.tile([P, 4], F32, tag="rstdb")
                if g == 0:
                    xts = xts0
                else:
                    xts = []
                    for i, t in enumerate(tiles):
                        xt = lnx.tile([P, C], F32, tag="xt")
                        nc.sync.dma_start(xt[:], x_in[t * P:(t + 1) * P, :])
                        xts.append(xt)
                if g == 0:
                    # group 0: per-tile LN -> transpose -> V chain so the PE
                    # starts ~12us earlier (V needs only its own tile)
                    for i, t in enumerate(tiles):
                        ln_stats(lnp, xts[i], mvb, i)
                        rsqrt_newton(lnp, mvb[:, i:i + 1, 1],
                                     rstdb[:, i:i + 1], 1, iters=2)
                        ht = htp.tile([P, C], F16, tag="ht")
                        ln_apply(xts[i], mvb, i, rstdb, ht)
                        nc.sync.dma_start(t_hT[:, t], ht[:], transpose=True)
                        v_tile(t)
                else:
                    # later groups: batched LN (one Newton chain), dense MM
                    # bursts keep the PE HAM-warm
                    for i, t in enumerate(tiles):
                        ln_stats(lnp, xts[i], mvb, i)
                    rsqrt_newton(lnp, mvb[:, :, 1], rstdb[:], 4, iters=2)
                    for i, t in enumerate(tiles):
                        ht = htp.tile([P, C], F16, tag="ht")
                        ln_apply(xts[i], mvb, i, rstdb, ht)
                        nc.sync.dma_start(t_hT[:, t], ht[:], transpose=True)
                    for t in tiles:
                        v_tile(t)

                gsl = slice(g * 512, (g + 1) * 512)
                # K^T for this group's tokens
                for f in range(CT):
                    t_wk = wst.tile([P, CT, P], F16, tag="wk")
                    nc.scalar.dma_start(t_wk[:], wk[f])
                    ps = psA.tile([P, 512], F32, tag="mm")
                    for c in range(CT):
                        nc.tensor.matmul(
                            ps[:], t_wk[:, c], t_hT[:, 4 * g:4 * g + 4, c, :],
                            start=(c == 0), stop=(c == CT - 1))
                    with nc.allow_low_precision(reason="fp16"):
                        nc.scalar.copy(out=t_KT[:, f, gsl], in_=ps[:])
                # Q^T for own-token groups
                if g < 2:
                    for f in range(CT):
                        t_wq = wst.tile([P, CT, P], F16, tag="wq")
                        nc.scalar.dma_start(t_wq[:], wq[f])
                        ps = psA.tile([P, 512], F32, tag="mm")
                        for c in range(CT):
                            nc.tensor.matmul(
                                ps[:], t_wq[:, c], t_hT[:, 4 * g:4 * g + 4, c, :],
                                start=(c == 0), stop=(c == CT - 1))
                        with nc.allow_low_precision(reason="fp16"):
                            nc.vector.tensor_scalar(
                                out=t_QT[:, f, gsl], in0=ps[:],
                                scalar1=t_qb[:, f:f + 1], scalar2=None, op0=OP.add)
                # attention c0 pair0: consume kt tiles as K/V complete
                if g >= 1:
                    attn_pass(0, 0, range(4 * (g - 1), 4 * g),
                              start=(g == 1), stop=False, state=c0p0)

            attn_pass(0, 0, range(12, 16), start=False, stop=True, state=c0p0)
            # wp needed first at proj(c0); load late so it never blocks
            # wk/wq/x streams
            nc.sync.dma_start(t_wp[:], wp[:].rearrange("c p n -> p c n"))

        s_hT.close()  # hT dead after QKV
        pool_g = top.enter_context(tc.tile_pool(name="gpool", bufs=1))
        t_g0 = pool_g.tile([P, FT, 512], F16)  # fc1/gelu staging chunk 0
        t_g1 = pool_g.tile([P, FT, 512], F16)  # fc1/gelu staging chunk 1
        t_gs = [t_g0, t_g1]

        # ---------------- attention c0 pairs 1-5 ----------------
        for pair in range(1, NPAIR):
            attn_pass(0, pair, range(NT), start=True, stop=True, state={})

        # ---------------- mlp helpers ----------------
        def mlp_head(c, lnp2):
            """proj + residual + LN2 + h2 transpose for chunk c."""
            mvb = lnp2.tile([P, 4, nc.vector.BN_AGGR_DIM], F32, tag="mvb2")
            for qt in range(4):
                tglob = 4 * c + qt
                xt = lnp2.tile([P, C], F32, tag="xres")
                nc.sync.dma_start(xt[:], x_in[tglob * P:(tglob + 1) * P, :])
                for nc2 in range(2):
                    ns = slice(nc2 * 384, (nc2 + 1) * 384)
                    ps = psA.tile([P, 512], F32, tag="mm")
                    for fc in range(CT):
                        nc.tensor.matmul(
                            ps[:, 0:384],
                            t_OT[:, c, fc, qt * P:(qt + 1) * P],
                            t_wp[:, fc, ns],
                            start=(fc == 0), stop=(fc == CT - 1))
                    nc.vector.tensor_tensor(
                        out=t_xo[:, c, qt, ns], in0=ps[:, 0:384], in1=xt[:, ns],
                        op=OP.add)
                if has_bpo:
                    nc.vector.tensor_tensor(
                        out=t_xo[:, c, qt, :], in0=t_xo[:, c, qt, :],
                        in1=t_bpo[:], op=OP.add)
                ln_stats(lnp2, t_xo[:, c, qt], mvb, qt)
            rstdb = lnp2.tile([P, 4], F32, tag="rstdb2")
            rsqrt_newton(lnp2, mvb[:, :, 1], rstdb[:], 4, iters=4)
            for qt in range(4):
                h2 = lnp2.tile([P, C], F16, tag="h2")
                ln_apply(t_xo[:, c, qt], mvb, qt, rstdb, h2)
                nc.sync.dma_start(t_h2T[:, c, qt], h2[:], transpose=True)

        def fc1_tiles(c, fs, w1st):
            weng = nc.sync
            for f in fs:
                t_w1 = w1st.tile([P, CT, P], F16, tag="w1")
                weng.dma_start(t_w1[:], w1[f])
                ps = psA.tile([P, 512], F32, tag="mm")
                for cc in range(CT):
                    nc.tensor.matmul(
                        ps[:], t_w1[:, cc], t_h2T[:, c, :, cc, :],
                        start=(cc == 0), stop=(cc == CT - 1))
                with nc.allow_low_precision(reason="fp16 staging"):
                    nc.vector.tensor_scalar(
                        out=t_gs[c][:, f, :], in0=ps[:],
                        scalar1=t_b1[:, f:f + 1], scalar2=None, op0=OP.add)

        def gelu_chunk(c, b=None):
            sl = slice(None) if b is None else slice(8 * b, 8 * b + 8)
            with nc.allow_low_precision(reason="fp16 gelu in place"):
                nc.scalar.activation(
                    out=t_gs[c][:, sl].rearrange("p f q -> p (f q)"),
                    in_=t_gs[c][:, sl].rearrange("p f q -> p (f q)"),
                    func=AF.Gelu)

        def fc2_chunk(c, w2st):
            weng = nc.sync
            NCH = 3
            FPC = FT // NCH
            for ch in range(NCH):
                t_w2 = w2st.tile([P, FPC, C], F16, tag="w2")
                weng.dma_start(
                    t_w2[:],
                    w2[ch * FPC:(ch + 1) * FPC].rearrange("f p n -> p f n"))
                for qt in range(4):
                    for nc2 in range(2):
                        ns = slice(nc2 * 384, (nc2 + 1) * 384)
                        ps = psA.tile([P, 512], F32, tag="mm")
                        for f in range(FPC):
                            nc.tensor.matmul(
                                ps[:, 0:384],
                                t_gs[c][:, ch * FPC + f, qt * P:(qt + 1) * P],
                                t_w2[:, f, ns],
                                start=(f == 0), stop=(f == FPC - 1))
                        nc.vector.tensor_tensor(
                            out=t_xo[:, c, qt, ns], in0=ps[:, 0:384],
                            in1=t_xo[:, c, qt, ns], op=OP.add)
            for qt in range(4):
                if has_bo:
                    nc.vector.tensor_tensor(
                        out=t_xo[:, c, qt, :], in0=t_xo[:, c, qt, :],
                        in1=t_bo[:], op=OP.add)
                tglob = 4 * c + qt
                nc.sync.dma_start(y[tglob * P:(tglob + 1) * P, :], t_xo[:, c, qt])

        # -------- seg3: attn(c1) interleaved with mlp(c0); seg4: mlp(c1) ----
        with ExitStack() as mlp_sec:
            lnp2 = mlp_sec.enter_context(tc.tile_pool(name="ln2", bufs=2))
            w1st = mlp_sec.enter_context(tc.tile_pool(name="w1st", bufs=5))
            w2st = mlp_sec.enter_context(
                tc.tile_pool(name="w2st", bufs=2 if fp8_pv else 1))

            mlp_head(0, lnp2)
            for pair in range(NPAIR):
                attn_pass(1, pair, range(NT), start=True, stop=True, state={})
                fc1_tiles(0, range(4 * pair, 4 * pair + 4), w1st)
            s_kqv.close()  # KT/QT/V no longer needed
            # gelu(c0) overlaps proj/fc1 of c1; gelu(c1) overlaps fc2(c0);
            # the two gelus share one activation-table era (1 switch total)
            mlp_head(1, lnp2)
            gelu_chunk(0)
            fc2_chunk(0, w2st)  # 31us of PE covering the LN2(c1) DVE chain
            for gb in range(3):
                fc1_tiles(1, range(8 * gb, 8 * gb + 8), w1st)
                gelu_chunk(1, gb)  # ScalarE idle here; batches hide under fc1
            fc2_chunk(1, w2st)

    nc.compile()
    return nc


def kernel(**inputs):
    global LAST_RESULT
    from concourse.bass_utils import run_bass_kernel_spmd

    x = np.asarray(inputs["x"], dtype=np.float32)
    ln1_g = np.asarray(inputs["ln1_g"], np.float32)
    ln1_b = np.asarray(inputs["ln1_b"], np.float32)
    w_qkv = np.asarray(inputs["w_qkv"], np.float32)
    w_proj = np.asarray(inputs["w_proj"], np.float32)
    b_proj = np.asarray(inputs["b_proj"], np.float32)
    ln2_g = np.asarray(inputs["ln2_g"], np.float32)
    ln2_b = np.asarray(inputs["ln2_b"], np.float32)
    w1 = np.asarray(inputs["w1"], np.float32)
    b1 = np.asarray(inputs["b1"], np.float32)
    w2 = np.asarray(inputs["w2"], np.float32)
    b2 = np.asarray(inputs["b2"], np.float32)

    # Fold LN affine params into the weights (exact algebra)
    w_qkv_eff = w_qkv * ln1_g[:, None]
    qkv_bias = ln1_b @ w_qkv                     # [3C]
    q_bias = qkv_bias[:C]                        # added to Q features
    vb = qkv_bias[2 * C:]                        # V bias -> folds into proj bias
    bpo = b_proj + vb @ w_proj                   # [C]
    w1_eff = w1 * ln2_g[:, None]
    b1_eff = b1 + ln2_b @ w1                     # [FF], applied pre-gelu
    has_bpo = bool(np.any(bpo != 0))
    has_bo = bool(np.any(b2 != 0))

    key = (has_bpo, has_bo, USE_FP8_PV)
    if key not in _CACHE:
        _CACHE[key] = _build(has_bpo, has_bo, USE_FP8_PV)
    nc = _CACHE[key]

    f16 = np.float16
    wq_h = np.ascontiguousarray(
        w_qkv_eff[:, :C].reshape(CT, P, CT, P).transpose(2, 1, 0, 3)).astype(f16)
    wk_h = np.ascontiguousarray(
        w_qkv_eff[:, C:2 * C].reshape(CT, P, CT, P).transpose(2, 1, 0, 3)).astype(f16)
    wv_h = np.ascontiguousarray(w_qkv_eff[:, 2 * C:].reshape(CT, P, C)).astype(f16)
    wp_h = np.ascontiguousarray(w_proj.reshape(CT, P, C)).astype(f16)
    w1_h = np.ascontiguousarray(
        w1_eff.reshape(CT, P, FT, P).transpose(2, 1, 0, 3)).astype(f16)
    w2_h = np.ascontiguousarray(w2.reshape(FT, P, C)).astype(f16)
    qb_h = np.ascontiguousarray(q_bias.reshape(CT, P).T)
    b1_h = np.ascontiguousarray(b1_eff.reshape(FT, P).T)

    shared = {
        "wq": wq_h, "wk": wk_h, "wv": wv_h, "wp": wp_h, "w1": w1_h, "w2": w2_h,
        "qb": qb_h, "b1v": b1_h,
        "bpo": bpo.astype(np.float32), "bo": b2.astype(np.float32),
        "onesc": np.ones((P, NT * HEADS), np.float16),
    }
    in_maps = []
    for core in range(8):
        b, half = core // 2, core % 2
        own = x[b, half * 1024:(half + 1) * 1024]
        other = x[b, (1 - half) * 1024:(2 - half) * 1024]
        x_c = np.ascontiguousarray(np.concatenate([own, other], axis=0))
        in_maps.append(dict(shared, x_in=x_c))

    trace = os.environ.get("KERNEL_TRACE", "0") == "1"
    res = run_bass_kernel_spmd(nc, in_maps, core_ids=list(range(8)), trace=trace)
    LAST_RESULT = res

    out = np.empty((B, N, C), dtype=np.float32)
    for core in range(8):
        b, half = core // 2, core % 2
        out[b, half * 1024:(half + 1) * 1024] = res.results[core]["y"]
    return out


# revision 27
# speedup vs baseline: 1.0157x; 1.0157x over previous
"""Trainium2 Bass kernel for a pre-norm transformer encoder block (v2).

Problem shapes (hardcoded): x [4, 2048, 768], 12 heads x 64, d_ff 3072.

Sharding: 8 cores, no collectives. Core c handles batch b = c // 2 and the
token half h = c % 2 (1024 "own" tokens). Each core receives the full 2048
tokens of its batch (own half first) so it can compute K/V locally; Q and
everything downstream (proj, MLP, output) run on its 1024 own tokens only.

v2 schedule (vs v1 phase-serial; ~1.5x target):
  - LayerNorm rstd via DVE Newton iteration (no ScalarE sqrt -> no activation
    table switches against exp/gelu; safe because var(x) ~ 1 here).
  - h -> h^T via DMA xbar transpose (SBUF->SBUF), PE transposes removed.
    hT layout [P, tile, CT, 128] so each transposed tile lands contiguous.
  - QKV in 4 token-groups of 512; attention chunk-0 head-pair-0 pass is
    interleaved into groups 1..3 (kt tiles consumed as K/V complete).
  - attention per 512-query chunk: 6 passes (1 head pair each) over 16 kt:
    S pair (PE row groups 0/64), exp [128,1024] on ScalarE, PV accumulate
    with the ones-column trick for softmax sums. PV emission is one kt
    behind exp so the PE never queues behind a pending exp.
    Optional fp8e4 DoubleRow PV (pt/V fp8, exp scaled 1/16 to stay < 240,
    two kt tiles contracted per matmul).
  - pipeline: attn(c0) -> [mlp-head(c0); {attn(c1, pair p); fc1(c0, 4f)}x6;
    gelu(c0); fc2(c0)] -> mlp(c1). ScalarE exp of chunk 1 overlaps chunk 0's
    MLP matmuls.
  - fc1 psum staged to SBUF f16 via tensor_scalar(+b1); ONE batched gelu per
    chunk (2 activation-table switches per chunk total).
"""

import os
import sys
import types

import numpy as np

# This image's antenv lacks ``axon_hooks``, so the boot shim can't register
# the NTFF-profiling hook and trace=True silently degrades. Provide the
# registry module with a lazily-built ctypes hook against libaxon_pjrt.so.
if "antenv.axon_hooks" not in sys.modules:
    _m = types.ModuleType("antenv.axon_hooks")
    _m._hook = None

    def _build_ctypes_hook():
        import contextlib
        import ctypes

        so_path = "/opt/axon/libaxon_pjrt.so"
        if not os.path.exists(so_path):
            return None
        lib = ctypes.CDLL(so_path)
        if not hasattr(lib, "axon_start_nrt_profile"):
            return None
        lib.axon_start_nrt_profile.argtypes = [
            ctypes.POINTER(ctypes.c_int64), ctypes.c_size_t]
        lib.axon_start_nrt_profile.restype = ctypes.c_int64
        lib.axon_stop_nrt_profile.argtypes = [ctypes.c_char_p]
        lib.axon_stop_nrt_profile.restype = ctypes.c_int64

        @contextlib.contextmanager
        def _hook(output_dir, device_ids):
            import jax
            jax.devices()
            if device_ids:
                ids = (ctypes.c_int64 * len(device_ids))(*device_ids)
                rc = lib.axon_start_nrt_profile(ids, len(device_ids))
            else:
                rc = lib.axon_start_nrt_profile(None, 0)
            if rc != 0:
                raise RuntimeError(f"axon_start_nrt_profile rc={rc}")
            try:
                yield
            finally:
                n = lib.axon_stop_nrt_profile(str(output_dir).encode())
                if n < 0:
                    raise RuntimeError(f"axon_stop_nrt_profile rc={n}")
                print(f"profile: {n} file(s) written to {output_dir}")

        return _hook

    def _set(h, _m=_m):
        _m._hook = h

    def _get(_m=_m):
        if _m._hook is None:
            _m._hook = _build_ctypes_hook()
        return _m._hook

    _m.set_axon_ntff_profile_hook = _set
    _m.get_axon_ntff_profile_hook = _get
    sys.modules["antenv.axon_hooks"] = _m

B, N, C = 4, 2048, 768
HEADS, HD = 12, 64
FF = 4 * C
P = 128
NT = N // P            # 16 token tiles (full context)
QT_ = (N // 2) // P    # 8 own token tiles
CT = C // P            # 6 feature tiles
FT = FF // P           # 24 ff tiles
NPAIR = HEADS // 2     # 6 head pairs (= CT: 128 features per pair)
LN_EPS = 1e-5

USE_FP8_PV = os.environ.get("KERNEL_FP8_PV", "1") == "1"
EXP_FP8_BIAS = -2.772588722239781  # -ln(16): keeps exp output <= ~19 << 240

_CACHE = {}
LAST_RESULT = None


def _build(has_bpo, has_bo, fp8_pv):
    import concourse.bass as bass
    import concourse.mybir as mybir
    import concourse.tile as tile
    from concourse import bacc
    from contextlib import ExitStack

    F32 = mybir.dt.float32
    F16 = mybir.dt.float16
    FP8 = mybir.dt.float8e4
    AF = mybir.ActivationFunctionType
    OP = mybir.AluOpType
    DR = mybir.MatmulPerfMode.DoubleRow

    nc = bacc.Bacc(None, target_bir_lowering=False)

    # ---- DRAM tensors ----
    x_in = nc.dram_tensor("x_in", [N, C], F32, kind="ExternalInput")
    wq = nc.dram_tensor("wq", [CT, P, CT, P], F16, kind="ExternalInput")
    wk = nc.dram_tensor("wk", [CT, P, CT, P], F16, kind="ExternalInput")
    wv = nc.dram_tensor("wv", [CT, P, C], F16, kind="ExternalInput")
    wp = nc.dram_tensor("wp", [CT, P, C], F16, kind="ExternalInput")
    w1 = nc.dram_tensor("w1", [FT, P, CT, P], F16, kind="ExternalInput")
    w2 = nc.dram_tensor("w2", [FT, P, C], F16, kind="ExternalInput")
    qb = nc.dram_tensor("qb", [P, CT], F32, kind="ExternalInput")
    b1v = nc.dram_tensor("b1v", [P, FT], F32, kind="ExternalInput")
    bpo = nc.dram_tensor("bpo", [C], F32, kind="ExternalInput")
    bo = nc.dram_tensor("bo", [C], F32, kind="ExternalInput")
    onesc = nc.dram_tensor("onesc", [P, NT * HEADS], F16, kind="ExternalInput")
    y = nc.dram_tensor("y", [N // 2, C], F32, kind="ExternalOutput")

    def bcast_rows(t):
        return bass.AP(tensor=t.tensor, offset=t.offset, ap=[[0, P], list(t.ap[0])])

    with tile.TileContext(nc) as tc, ExitStack() as top:
        consts = top.enter_context(tc.tile_pool(name="consts", bufs=1))
        t_qb = consts.tile([P, CT], F32)
        t_b1 = consts.tile([P, FT], F32)
        t_eps = consts.tile([P, 1], F32)
        nc.vector.memset(t_eps[:], LN_EPS)
        t_eb = consts.tile([P, 1], F32)
        nc.vector.memset(t_eb[:], EXP_FP8_BIAS)
        t_bpo = t_bo = None
        if has_bpo:
            t_bpo = consts.tile([P, C], F32)
        if has_bo:
            t_bo = consts.tile([P, C], F32)

        # ---- persistent SBUF state ----
        s_kqv = ExitStack()   # KT/QT/V: freed after attention c1
        s_hT = ExitStack()    # hT: freed after QKV
        s_big = ExitStack()   # OT/xo/h2T/g: until end
        top.enter_context(s_big)

        pool_kqv = s_kqv.enter_context(tc.tile_pool(name="kqv", bufs=1, side="right"))
        t_KT = pool_kqv.tile([P, NPAIR, N], F16)       # K^T feature-major
        t_QT = pool_kqv.tile([P, NPAIR, N // 2], F16)  # Q^T own tokens
        if fp8_pv:
            # [P, kt-pair, j, head, 68]: DoubleRow lhsT; col 64 = ones
            t_V = pool_kqv.tile([P, NT // 2, 2, HEADS, 68], FP8)
        else:
            t_V = pool_kqv.tile([P, NT, HEADS, HD + 1], F16)

        pool_hT = s_hT.enter_context(tc.tile_pool(name="hT", bufs=1, side="right"))
        t_hT = pool_hT.tile([P, NT, CT, P], F16)

        pool_big = s_big.enter_context(tc.tile_pool(name="big", bufs=1))
        t_OT = pool_big.tile([P, 2, NPAIR, 512], F16)   # O^T per chunk
        t_xo = pool_big.tile([P, 2, 4, C], F32)         # residual accum
        t_h2T = pool_big.tile([P, 2, 4, CT, P], F16)
        t_g = None  # fc1/gelu staging; allocated after hT frees its space

        wpool = top.enter_context(tc.tile_pool(name="wlong", bufs=1))
        t_wp = wpool.tile([P, CT, C], F16)

        # ---- PSUM pools (8 banks total: psA 2 + psS 4 + psO 2) ----
        psA = top.enter_context(tc.tile_pool(name="psA", bufs=2, space="PSUM"))
        psS = top.enter_context(tc.tile_pool(name="psS", bufs=2, space="PSUM"))
        psO = top.enter_context(tc.tile_pool(name="psO", bufs=1, space="PSUM"))

        ptp = top.enter_context(tc.tile_pool(name="pt", bufs=3))
        rbp = top.enter_context(tc.tile_pool(name="rb", bufs=1))

        nc.sync.dma_start(t_qb[:], qb[:])
        nc.sync.dma_start(t_b1[:], b1v[:])
        if has_bpo:
            nc.sync.dma_start(t_bpo[:], bcast_rows(bpo[:]))
        if has_bo:
            nc.sync.dma_start(t_bo[:], bcast_rows(bo[:]))

        def rsqrt_newton(pool, var_ap, out, n, iters):
            # out [P, n] f32 = 1/sqrt(var + eps). Newton from y0 = 1/(var+eps)
            # converges monotonically from below for var+eps > 1/3 (true here:
            # LN inputs have variance ~1).
            v = pool.tile([P, n], F32, tag=f"lnv{n}")
            nc.vector.tensor_scalar(
                out=v[:], in0=var_ap, scalar1=t_eps[:, 0:1], scalar2=None,
                op0=OP.add)
            nc.vector.reciprocal(out=out, in_=v[:])
            t = pool.tile([P, n], F32, tag=f"lnt{n}")
            for _ in range(iters):
                nc.vector.tensor_tensor(out=t[:], in0=out, in1=out, op=OP.mult)
                nc.vector.tensor_tensor(out=t[:], in0=t[:], in1=v[:], op=OP.mult)
                nc.vector.tensor_scalar(
                    out=t[:], in0=t[:], scalar1=-0.5, scalar2=1.5,
                    op0=OP.mult, op1=OP.add)
                nc.vector.tensor_tensor(out=out, in0=out, in1=t[:], op=OP.mult)

        def ln_stats(pool, xt, mvb, i):
            stats = pool.tile([P, 3, nc.vector.BN_STATS_DIM], F32, tag="ln_stats")
            for sg in range(3):
                nc.vector.bn_stats(out=stats[:, sg], in_=xt[:, sg * 256:(sg + 1) * 256])
            nc.vector.bn_aggr(out=mvb[:, i], in_=stats[:])

        def ln_apply(xt, mvb, i, rstdb, ht):
            with nc.allow_low_precision(reason="fp16 for matmul input"):
                nc.vector.tensor_scalar(
                    out=ht[:], in0=xt[:], scalar1=mvb[:, i, 0:1],
                    scalar2=rstdb[:, i:i + 1], op0=OP.subtract, op1=OP.mult)

        # ---------------- attention pass (one head pair) ----------------
        def attn_pass(chunk, pair, kts, start, stop, state):
            qs = slice(chunk * 512, (chunk + 1) * 512)
            if start:
                state["pso"] = [
                    psO.tile([HD + 1, 512], F32, tag=f"o{sub}",
                             name=f"pso{sub}_{chunk}_{pair}")
                    for sub in range(2)
                ]
                state["pend"] = None
            pso = state["pso"]

            def emit_pv_f16(kt, pt):
                for sub in range(2):
                    nc.tensor.matmul(
                        pso[sub][:], t_V[:, kt, 2 * pair + sub, :],
                        pt[:, sub * 512:(sub + 1) * 512],
                        start=(kt == 0), stop=(kt == NT - 1))

            def emit_pv_fp8(kp, pt8):
                for sub in range(2):
                    nc.tensor.matmul(
                        pso[sub][:],
                        t_V[:, kp, :, 2 * pair + sub, 0:HD + 1],
                        pt8[:, :, sub * 512:(sub + 1) * 512],
                        start=(kp == 0), stop=(kp == NT // 2 - 1),
                        perf_mode=DR)

            for kt in kts:
                ps = psS.tile([P, 1024], F32, tag="s")
                for sub in range(2):
                    off = sub * HD
                    nc.tensor.matmul(
                        ps[:, sub * 512:(sub + 1) * 512],
                        t_KT[off:off + HD, pair, kt * P:(kt + 1) * P],
                        t_QT[off:off + HD, pair, qs], start=True, stop=True)
                if fp8_pv:
                    j = kt % 2
                    if j == 0:
                        state["pt8"] = ptp.tile(
                            [P, 2, 1024], FP8, tag="pt8",
                            name=f"pt8_{chunk}_{pair}_{kt}")
                    pt8 = state["pt8"]
                    nc.scalar.activation(
                        out=pt8[:, j], in_=ps[:], func=AF.Exp, scale=0.125,
                        bias=t_eb[:])
                    if j == 1:
                        if state["pend"] is not None:
                            emit_pv_fp8(*state["pend"])
                        state["pend"] = (kt // 2, pt8)
                else:
                    pt = ptp.tile([P, 1024], F16, tag="pt")
                    nc.scalar.activation(
                        out=pt[:], in_=ps[:], func=AF.Exp, scale=0.125)
                    if state["pend"] is not None:
                        emit_pv_f16(*state["pend"])
                    state["pend"] = (kt, pt)

            if stop:
                if state["pend"] is not None:
                    if fp8_pv:
                        emit_pv_fp8(*state["pend"])
                    else:
                        emit_pv_f16(*state["pend"])
                    state["pend"] = None
                for sub in range(2):
                    off = sub * HD
                    sums = rbp.tile([1, 512], F32, tag="sums")
                    nc.vector.tensor_copy(out=sums[:], in_=pso[sub][HD:HD + 1, :])
                    r32 = rbp.tile([1, 512], F32, tag="r32")
                    nc.vector.reciprocal_approx_fast(out=r32[:], in_=sums[:])
                    rb = rbp.tile([HD, 512], F32, tag="rb")
                    nc.gpsimd.partition_broadcast(rb[:], r32[:])
                    with nc.allow_low_precision(reason="fp16 matmul input"):
                        nc.vector.tensor_tensor(
                            out=t_OT[off:off + HD, chunk, pair, :],
                            in0=pso[sub][:HD, :], in1=rb[:], op=OP.mult)

        # -------- QKV over 4 token groups (+ attn c0 pair 0 interleaved) ----
        with ExitStack() as qkv_sec:
            lnx = qkv_sec.enter_context(tc.tile_pool(name="lnx", bufs=6))
            lnp = qkv_sec.enter_context(tc.tile_pool(name="ln1", bufs=2))
            htp = qkv_sec.enter_context(tc.tile_pool(name="htp", bufs=2))
            wst = qkv_sec.enter_context(tc.tile_pool(name="wst", bufs=4))
            wvp = qkv_sec.enter_context(tc.tile_pool(name="wv", bufs=1))

            xts0 = []
            for t in range(4):
                xt = lnx.tile([P, C], F32, tag="xt", name=f"xt0_{t}")
                nc.sync.dma_start(xt[:], x_in[t * P:(t + 1) * P, :])
                xts0.append(xt)
            t_wv = wvp.tile([P, CT, C], F16, tag="wv")
            nc.sync.dma_start(t_wv[:], wv[:].rearrange("c p n -> p c n"))
            if fp8_pv:
                for kp in range(NT // 2):
                    for j in range(2):
                        nc.gpsimd.memset(t_V[:, kp, j, :, HD:HD + 1], 1.0)
            else:
                nc.sync.dma_start(
                    t_V[:, :, :, HD:HD + 1],
                    onesc[:].rearrange("p (t h) -> p t h", t=NT)[:, :, :, None])

            c0p0 = {}  # attention chunk-0 pair-0 state (split across groups)

            def v_tile(t):
                for nc2 in range(2):
                    ps = psA.tile([P, 512], F32, tag="mm")
                    for c in range(CT):
                        nc.tensor.matmul(
                            ps[:, 0:384], t_hT[:, t, c, :],
                            t_wv[:, c, nc2 * 384:(nc2 + 1) * 384],
                            start=(c == 0), stop=(c == CT - 1))
                    with nc.allow_low_precision(reason="fp16/fp8"):
                        if fp8_pv:
                            nc.vector.tensor_copy(
                                out=t_V[:, t // 2, t % 2,
                                        6 * nc2:6 * nc2 + 6, 0:HD],
                                in_=ps[:, 0:384].rearrange(
                                    "p (h d) -> p h d", d=HD))
                        else:
                            nc.vector.tensor_copy(
                                out=t_V[:, t, 6 * nc2:6 * nc2 + 6, :HD],
                                in_=ps[:, 0:384].rearrange(
                                    "p (h d) -> p h d", d=HD))

            for g in range(4):  # token groups of 512
                tiles = list(range(4 * g, 4 * g + 4))
                mvb = lnp.tile([P, 4, nc.vector.BN_AGGR_DIM], F32, tag="mvb")
                rstdb = lnp.tile([P, 4], F32, tag="rstdb")
                if g == 0:
                    xts = xts0
                else:
                    xts = []
                    for i, t in enumerate(tiles):
                        xt = lnx.tile([P, C], F32, tag="xt")
                        nc.sync.dma_start(xt[:], x_in[t * P:(t + 1) * P, :])
                        xts.append(xt)
                if g == 0:
                    # group 0: per-tile LN -> transpose -> V chain so the PE
                    # starts ~12us earlier (V needs only its own tile)
                    for i, t in enumerate(tiles):
                        ln_stats(lnp, xts[i], mvb, i)
                        rsqrt_newton(lnp, mvb[:, i:i + 1, 1],
                                     rstdb[:, i:i + 1], 1, iters=2)
                        ht = htp.tile([P, C], F16, tag="ht")
                        ln_apply(xts[i], mvb, i, rstdb, ht)
                        teng = nc.sync if t % 2 == 0 else nc.scalar
                        teng.dma_start(t_hT[:, t], ht[:], transpose=True)
                        v_tile(t)
                else:
                    # later groups: batched LN (one Newton chain), dense MM
                    # bursts keep the PE HAM-warm
                    for i, t in enumerate(tiles):
                        ln_stats(lnp, xts[i], mvb, i)
                    rsqrt_newton(lnp, mvb[:, :, 1], rstdb[:], 4, iters=2)
                    for i, t in enumerate(tiles):
                        ht = htp.tile([P, C], F16, tag="ht")
                        ln_apply(xts[i], mvb, i, rstdb, ht)
                        teng = nc.sync if t % 2 == 0 else nc.scalar
                        teng.dma_start(t_hT[:, t], ht[:], transpose=True)
                    for t in tiles:
                        v_tile(t)

                gsl = slice(g * 512, (g + 1) * 512)
                # K^T for this group's tokens
                for f in range(CT):
                    t_wk = wst.tile([P, CT, P], F16, tag="wk")
                    nc.scalar.dma_start(t_wk[:], wk[f])
                    ps = psA.tile([P, 512], F32, tag="mm")
                    for c in range(CT):
                        nc.tensor.matmul(
                            ps[:], t_wk[:, c], t_hT[:, 4 * g:4 * g + 4, c, :],
                            start=(c == 0), stop=(c == CT - 1))
                    with nc.allow_low_precision(reason="fp16"):
                        nc.vector.tensor_copy(out=t_KT[:, f, gsl], in_=ps[:])
                # Q^T for own-token groups
                if g < 2:
                    for f in range(CT):
                        t_wq = wst.tile([P, CT, P], F16, tag="wq")
                        nc.scalar.dma_start(t_wq[:], wq[f])
                        ps = psA.tile([P, 512], F32, tag="mm")
                        for c in range(CT):
                            nc.tensor.matmul(
                                ps[:], t_wq[:, c], t_hT[:, 4 * g:4 * g + 4, c, :],
                                start=(c == 0), stop=(c == CT - 1))
                        with nc.allow_low_precision(reason="fp16"):
                            nc.vector.tensor_scalar(
                                out=t_QT[:, f, gsl], in0=ps[:],
                                scalar1=t_qb[:, f:f + 1], scalar2=None, op0=OP.add)
                # attention c0 pair0: consume kt tiles as K/V complete
                if g >= 1:
                    attn_pass(0, 0, range(4 * (g - 1), 4 * g),
                              start=(g == 1), stop=False, state=c0p0)

            attn_pass(0, 0, range(12, 16), start=False, stop=True, state=c0p0)
            # wp needed first at proj(c0); load late so it never blocks
            # wk/wq/x streams
            nc.sync.dma_start(t_wp[:], wp[:].rearrange("c p n -> p c n"))

        s_hT.close()  # hT dead after QKV
        pool_g = top.enter_context(tc.tile_pool(name="gpool", bufs=1))
        t_g0 = pool_g.tile([P, FT, 512], F16)  # fc1/gelu staging chunk 0
        t_g1 = pool_g.tile([P, FT, 512], F16)  # fc1/gelu staging chunk 1
        t_gs = [t_g0, t_g1]

        # ---------------- attention c0 pairs 1-5 ----------------
        for pair in range(1, NPAIR):
            attn_pass(0, pair, range(NT), start=True, stop=True, state={})

        # ---------------- mlp helpers ----------------
        def mlp_head(c, lnp2):
            """proj + residual + LN2 + h2 transpose for chunk c."""
            mvb = lnp2.tile([P, 4, nc.vector.BN_AGGR_DIM], F32, tag="mvb2")
            for qt in range(4):
                tglob = 4 * c + qt
                xt = lnp2.tile([P, C], F32, tag="xres")
                nc.sync.dma_start(xt[:], x_in[tglob * P:(tglob + 1) * P, :])
                for nc2 in range(2):
                    ns = slice(nc2 * 384, (nc2 + 1) * 384)
                    ps = psA.tile([P, 512], F32, tag="mm")
                    for fc in range(CT):
                        nc.tensor.matmul(
                            ps[:, 0:384],
                            t_OT[:, c, fc, qt * P:(qt + 1) * P],
                            t_wp[:, fc, ns],
                            start=(fc == 0), stop=(fc == CT - 1))
                    nc.vector.tensor_tensor(
                        out=t_xo[:, c, qt, ns], in0=ps[:, 0:384], in1=xt[:, ns],
                        op=OP.add)
                if has_bpo:
                    nc.vector.tensor_tensor(
                        out=t_xo[:, c, qt, :], in0=t_xo[:, c, qt, :],
                        in1=t_bpo[:], op=OP.add)
                ln_stats(lnp2, t_xo[:, c, qt], mvb, qt)
            rstdb = lnp2.tile([P, 4], F32, tag="rstdb2")
            rsqrt_newton(lnp2, mvb[:, :, 1], rstdb[:], 4, iters=4)
            for qt in range(4):
                h2 = lnp2.tile([P, C], F16, tag="h2")
                ln_apply(t_xo[:, c, qt], mvb, qt, rstdb, h2)
                nc.sync.dma_start(t_h2T[:, c, qt], h2[:], transpose=True)

        def fc1_tiles(c, fs, w1st):
            weng = nc.sync
            for f in fs:
                t_w1 = w1st.tile([P, CT, P], F16, tag="w1")
                weng.dma_start(t_w1[:], w1[f])
                ps = psA.tile([P, 512], F32, tag="mm")
                for cc in range(CT):
                    nc.tensor.matmul(
                        ps[:], t_w1[:, cc], t_h2T[:, c, :, cc, :],
                        start=(cc == 0), stop=(cc == CT - 1))
                with nc.allow_low_precision(reason="fp16 staging"):
                    nc.vector.tensor_scalar(
                        out=t_gs[c][:, f, :], in0=ps[:],
                        scalar1=t_b1[:, f:f + 1], scalar2=None, op0=OP.add)

        def gelu_chunk(c, b=None):
            sl = slice(None) if b is None else slice(8 * b, 8 * b + 8)
            with nc.allow_low_precision(reason="fp16 gelu in place"):
                nc.scalar.activation(
                    out=t_gs[c][:, sl].rearrange("p f q -> p (f q)"),
                    in_=t_gs[c][:, sl].rearrange("p f q -> p (f q)"),
                    func=AF.Gelu)

        def fc2_chunk(c, w2st):
            weng = nc.sync
            NCH = 3
            FPC = FT // NCH
            for ch in range(NCH):
                t_w2 = w2st.tile([P, FPC, C], F16, tag="w2")
                weng.dma_start(
                    t_w2[:],
                    w2[ch * FPC:(ch + 1) * FPC].rearrange("f p n -> p f n"))
                for qt in range(4):
                    for nc2 in range(2):
                        ns = slice(nc2 * 384, (nc2 + 1) * 384)
                        ps = psA.tile([P, 512], F32, tag="mm")
                        for f in range(FPC):
                            nc.tensor.matmul(
                                ps[:, 0:384],
                                t_gs[c][:, ch * FPC + f, qt * P:(qt + 1) * P],
                                t_w2[:, f, ns],
                                start=(f == 0), stop=(f == FPC - 1))
                        nc.vector.tensor_tensor(
                            out=t_xo[:, c, qt, ns], in0=ps[:, 0:384],
                            in1=t_xo[:, c, qt, ns], op=OP.add)
            for qt in range(4):
                if has_bo:
                    nc.vector.tensor_tensor(
                        out=t_xo[:, c, qt, :], in0=t_xo[:, c, qt, :],
                        in1=t_bo[:], op=OP.add)
                tglob = 4 * c + qt
                nc.sync.dma_start(y[tglob * P:(tglob + 1) * P, :], t_xo[:, c, qt])

        # -------- seg3: attn(c1) interleaved with mlp(c0); seg4: mlp(c1) ----
        with ExitStack() as mlp_sec:
            lnp2 = mlp_sec.enter_context(tc.tile_pool(name="ln2", bufs=2))
            w1st = mlp_sec.enter_context(tc.tile_pool(name="w1st", bufs=5))
            w2st = mlp_sec.enter_context(
                tc.tile_pool(name="w2st", bufs=2 if fp8_pv else 1))

            mlp_head(0, lnp2)
            for pair in range(NPAIR):
                attn_pass(1, pair, range(NT), start=True, stop=True, state={})
                fc1_tiles(0, range(4 * pair, 4 * pair + 4), w1st)
            s_kqv.close()  # KT/QT/V no longer needed
            # gelu(c0) overlaps proj/fc1 of c1; gelu(c1) overlaps fc2(c0);
            # the two gelus share one activation-table era (1 switch total)
            mlp_head(1, lnp2)
            gelu_chunk(0)
            fc2_chunk(0, w2st)  # 31us of PE covering the LN2(c1) DVE chain
            for gb in range(3):
                fc1_tiles(1, range(8 * gb, 8 * gb + 8), w1st)
                gelu_chunk(1, gb)  # ScalarE idle here; batches hide under fc1
            fc2_chunk(1, w2st)

    nc.compile()
    return nc


def kernel(**inputs):
    global LAST_RESULT
    from concourse.bass_utils import run_bass_kernel_spmd

    x = np.asarray(inputs["x"], dtype=np.float32)
    ln1_g = np.asarray(inputs["ln1_g"], np.float32)
    ln1_b = np.asarray(inputs["ln1_b"], np.float32)
    w_qkv = np.asarray(inputs["w_qkv"], np.float32)
    w_proj = np.asarray(inputs["w_proj"], np.float32)
    b_proj = np.asarray(inputs["b_proj"], np.float32)
    ln2_g = np.asarray(inputs["ln2_g"], np.float32)
    ln2_b = np.asarray(inputs["ln2_b"], np.float32)
    w1 = np.asarray(inputs["w1"], np.float32)
    b1 = np.asarray(inputs["b1"], np.float32)
    w2 = np.asarray(inputs["w2"], np.float32)
    b2 = np.asarray(inputs["b2"], np.float32)

    # Fold LN affine params into the weights (exact algebra)
    w_qkv_eff = w_qkv * ln1_g[:, None]
    qkv_bias = ln1_b @ w_qkv                     # [3C]
    q_bias = qkv_bias[:C]                        # added to Q features
    vb = qkv_bias[2 * C:]                        # V bias -> folds into proj bias
    bpo = b_proj + vb @ w_proj                   # [C]
    w1_eff = w1 * ln2_g[:, None]
    b1_eff = b1 + ln2_b @ w1                     # [FF], applied pre-gelu
    has_bpo = bool(np.any(bpo != 0))
    has_bo = bool(np.any(b2 != 0))

    key = (has_bpo, has_bo, USE_FP8_PV)
    if key not in _CACHE:
        _CACHE[key] = _build(has_bpo, has_bo, USE_FP8_PV)
    nc = _CACHE[key]

    f16 = np.float16
    wq_h = np.ascontiguousarray(
        w_qkv_eff[:, :C].reshape(CT, P, CT, P).transpose(2, 1, 0, 3)).astype(f16)
    wk_h = np.ascontiguousarray(
        w_qkv_eff[:, C:2 * C].reshape(CT, P, CT, P).transpose(2, 1, 0, 3)).astype(f16)
    wv_h = np.ascontiguousarray(w_qkv_eff[:, 2 * C:].reshape(CT, P, C)).astype(f16)
    wp_h = np.ascontiguousarray(w_proj.reshape(CT, P, C)).astype(f16)
    w1_h = np.ascontiguousarray(
        w1_eff.reshape(CT, P, FT, P).transpose(2, 1, 0, 3)).astype(f16)
    w2_h = np.ascontiguousarray(w2.reshape(FT, P, C)).astype(f16)
    qb_h = np.ascontiguousarray(q_bias.reshape(CT, P).T)
    b1_h = np.ascontiguousarray(b1_eff.reshape(FT, P).T)

    shared = {
        "wq": wq_h, "wk": wk_h, "wv": wv_h, "wp": wp_h, "w1": w1_h, "w2": w2_h,
        "qb": qb_h, "b1v": b1_h,
        "bpo": bpo.astype(np.float32), "bo": b2.astype(np.float32),
        "onesc": np.ones((P, NT * HEADS), np.float16),
    }
    in_maps = []
    for core in range(8):
        b, half = core // 2, core % 2
        own = x[b, half * 1024:(half + 1) * 1024]
        other = x[b, (1 - half) * 1024:(2 - half) * 1024]
        x_c = np.ascontiguousarray(np.concatenate([own, other], axis=0))
        in_maps.append(dict(shared, x_in=x_c))

    trace = os.environ.get("KERNEL_TRACE", "0") == "1"
    res = run_bass_kernel_spmd(nc, in_maps, core_ids=list(range(8)), trace=trace)
    LAST_RESULT = res

    out = np.empty((B, N, C), dtype=np.float32)
    for core in range(8):
        b, half = core // 2, core % 2
        out[b, half * 1024:(half + 1) * 1024] = res.results[core]["y"]
    return out
